# revision 10
# baseline (speedup 1.0000x reference)
"""DRNN (dilated ResLSTM stack) Trainium2 Bass kernel.

Strategy: data-parallel over batch B=32 across 8 cores (4 examples/core).
Layers run sequentially per core; the time scan is fully unrolled.

Per scan step (layer l, rate r, T_l = 512/r steps, Bd = 4*r dilated batch):
  gates psum [Bd, 1024] = [ifo(768) | cellpre(256)]
    accumulated as lhsT.T @ Wcomb with lhsT chunks = [hT; cT; xT; ones]
    (state kept TRANSPOSED in SBUF; x comes from the previous layer's stored
     transposed outputs, biases ride as a K=1 ones chunk; fp32r matmuls)
  elementwise: sigmoid(ifo) + tanh on ACT, gate algebra on DVE
  hy/cy transposed back via 4 PE-transposes -> state for the next step;
  hy also DMA'd to DRAM (next layer's skip input + final outputs).
"""

import os
from contextlib import ExitStack

import ml_dtypes
import numpy as np

import concourse.bass as bass
import concourse.mybir as mybir
import concourse.tile as tile
from concourse.bass_utils import run_bass_kernel_spmd
from concourse.masks import make_identity

T_FULL = 512
B = 32
NIN = 64
HID = 256
RATES = [1, 2, 4, 8]
NCORES = 8
BPC = B // NCORES  # 4 examples per core
GATES = 3 * HID  # 768
NTOT = GATES + HID  # 1024

F32 = mybir.dt.float32
BF16 = mybir.dt.bfloat16
AF = mybir.ActivationFunctionType
OP = mybir.AluOpType


def _dims(T):
    out = []
    for r in RATES:
        out.append((r, T // r, BPC * r))
    return out


def _r32(ap):
    return ap.bitcast(mybir.dt.float32r)


def _split_multiwaits(nc):
    """This toolchain's walrus accepts only one sem-wait per instruction.
    Hoist extra waits onto same-engine NoOps (engines drain their streams in
    order, so a prefixed wait is equivalent). DMA descriptors are left alone:
    their waits are consumed per-descriptor by the DGE, and hoisting them onto
    the SP sequencer can deadlock unrelated queues."""
    f = nc.m.functions[0]
    for bb in f.blocks:
        new_list, changed = [], False
        for inst in bb.instructions:
            si = inst.sync_info
            if si is not None and si.on_wait is not None and len(si.on_wait) > 1:
                waits = list(si.on_wait)
                if isinstance(inst, (mybir.InstDMACopy, mybir.InstDMA)):
                    # Keep the compute-engine (data-producer) wait embedded in
                    # the descriptor; hoist DMA-queue ordering waits onto the
                    # SP sequencer. Their producing DMAs precede this point in
                    # SP program order, so they complete regardless of the SP
                    # stall -> no deadlock.
                    keep = None
                    for w in waits:
                        if not w.ant_name.startswith("DMA"):
                            keep = w
                    if keep is None:
                        keep = waits[-1]
                    hoist = [w for w in waits if w is not keep]
                    if any(not w.ant_name.startswith("DMA") for w in hoist):
                        raise RuntimeError(
                            f"{inst.name}: DMA with >1 engine waits "
                            f"{[w.ant_name for w in waits]}"
                        )
                else:
                    keep = waits[-1]
                    hoist = waits[:-1]
                for k, w in enumerate(hoist):
                    nop = mybir.InstNoOp(name=f"{inst.name}_w{k}", ins=[], outs=[])
                    nop.engine = inst.engine
                    nop.sync_info = type(si)(on_wait=[w], on_update=[])
                    new_list.append(nop)
                si.on_wait = [keep]
                inst.sync_info = si
                changed = True
            new_list.append(inst)
        if changed:
            cur = bb.instructions
            cur.clear()
            cur.extend(new_list)


def _build_bass(T=T_FULL):
    dims = _dims(T)
    nc = bass.Bass(trn_type="TRN2")

    xT0 = nc.dram_tensor("xT0", [NIN, T, BPC], BF16, kind="ExternalInput")
    krows = [2 * HID + NIN + 1] + [2 * HID + HID + 1] * 3  # 577, 769, 769, 769
    wcs = [
        nc.dram_tensor(f"wc{l}", [krows[l], NTOT], BF16, kind="ExternalInput")
        for l in range(4)
    ]
    wir = nc.dram_tensor("wir", [NIN, HID], BF16, kind="ExternalInput")
    hs_out = [
        nc.dram_tensor(f"hs{l}", [Tl * Bd, HID], F32, kind="ExternalOutput")
        for l, (r, Tl, Bd) in enumerate(dims)
    ]

    with tile.TileContext(nc) as tc, ExitStack() as ctx:
        ep = ctx.enter_context
        wpool = ep(tc.tile_pool(name="w", bufs=2))
        bpool = ep(tc.tile_pool(name="wb", bufs=2))
        hyTp = ep(tc.tile_pool(name="hyT", bufs=1))
        constp = ep(tc.tile_pool(name="const", bufs=1))
        cTp = ep(tc.tile_pool(name="cT", bufs=3))
        cxp = ep(tc.tile_pool(name="cx", bufs=3))
        ewp = ep(tc.tile_pool(name="ew", bufs=2))
        hyp = ep(tc.tile_pool(name="hy", bufs=3))
        pg_pool = ep(tc.tile_pool(name="pg", bufs=2, space="PSUM"))
        ps_pool = ep(tc.tile_pool(name="ps", bufs=2, space="PSUM"))
        pt_pool = ep(tc.tile_pool(name="pt", bufs=2, space="PSUM"))

        ident = constp.tile([128, 128], F32, tag="ident")
        make_identity(nc, ident)
        identb = constp.tile([128, 128], BF16, tag="identb")
        make_identity(nc, identb)
        ones = constp.tile([1, 32], BF16, tag="ones")
        nc.vector.memset(ones, 1.0)
        zT = constp.tile([128, 2, 32], BF16, tag="zT")
        nc.vector.memset(zT, 0.0)
        zcx = constp.tile([32, HID], F32, tag="zcx")
        nc.vector.memset(zcx, 0.0)

        xT0_sb = constp.tile([NIN, T, BPC], BF16, tag="xT0")
        nc.sync.dma_start(xT0_sb, xT0[:, :, :])
        wir_sb = constp.tile([NIN, HID], BF16, tag="wir")
        nc.sync.dma_start(wir_sb, wir[:, :])

        hyT_arr = [
            hyTp.tile([128, 2, Tl, Bd], BF16, tag=f"hyT{l}", name=f"hyT{l}")
            for l, (r, Tl, Bd) in enumerate(dims)
        ]

        for l, (r, Tl, Bd) in enumerate(dims):
            C = NIN if l == 0 else HID
            K = krows[l]
            # weight chunk tiles: 4 hc chunks + x chunks
            nxc = 1 if l == 0 else 2
            wt = [wpool.tile([128, NTOT], BF16, tag=f"w{i}", name=f"w{l}_{i}") for i in range(4 + nxc)]
            for i in range(4):
                nc.sync.dma_start(wt[i], wcs[l][i * 128 : (i + 1) * 128, :])
            if l == 0:
                nc.sync.dma_start(wt[4][:NIN, :], wcs[l][512 : 512 + NIN, :])
            else:
                nc.sync.dma_start(wt[4], wcs[l][512:640, :])
                nc.sync.dma_start(wt[5], wcs[l][640:768, :])
            wb = bpool.tile([1, NTOT], BF16, tag="wb")
            nc.sync.dma_start(wb, wcs[l][K - 1 : K, :])

            prev_cT = None
            prev_cx = None
            Bdp = Bd // 2  # previous layer's dilated batch (l>0)

            for t in range(Tl):
                # ---- gate matmuls: psum[Bd, 1024] ----
                pg = pg_pool.tile([128, NTOT], F32, tag="pg")
                # lhsT chunk list: (ap, wtile_or_bias)
                chunks = []
                for i in range(2):  # h chunks
                    src = (
                        hyT_arr[l][:, i, t - 1, :]
                        if t > 0
                        else zT[:, i, :Bd]
                    )
                    chunks.append((src, wt[i]))
                for i in range(2):  # c chunks
                    src = prev_cT[:, i, :Bd] if t > 0 else zT[:, i, :Bd]
                    chunks.append((src, wt[2 + i]))
                if l == 0:
                    chunks.append((xT0_sb[:, t, :], wt[4][:NIN, :]))
                else:
                    for i in range(2):
                        chunks.append(
                            (hyT_arr[l - 1][:, i, 2 * t : 2 * t + 2, :], wt[4 + i])
                        )
                for half in (0, 1):
                    s0, s1 = half * 512, half * 512 + 512
                    nc.tensor.matmul(
                        pg[:Bd, s0:s1],
                        ones[:1, :Bd],
                        wb[:1, s0:s1],
                        start=True,
                        stop=False,
                    )
                    for j, (src, w) in enumerate(chunks):
                        nc.tensor.matmul(
                            pg[:Bd, s0:s1],
                            src,
                            w[:, s0:s1],
                            start=False,
                            stop=(j == len(chunks) - 1),
                        )

                # ---- skip connection ----
                if l == 0:
                    ps = ps_pool.tile([128, HID], F32, tag="ps")
                    nc.tensor.matmul(
                        ps[:Bd, :],
                        xT0_sb[:, t, :],
                        wir_sb,
                        start=True,
                        stop=True,
                    )
                    skip_ap = ps[:Bd, :]
                else:
                    ps = ps_pool.tile([128, HID], BF16, tag="ps")
                    for i in range(2):
                        nc.tensor.transpose(
                            ps[:Bd, i * 128 : (i + 1) * 128],
                            hyT_arr[l - 1][:, i, 2 * t : 2 * t + 2, :],
                            identb,
                        )
                    skip_ap = ps[:Bd, :]

                # ---- elementwise ----
                sifo = ewp.tile([32, GATES], F32, tag="sifo")
                nc.scalar.activation(sifo[:Bd, :], pg[:Bd, 0:GATES], AF.Sigmoid)
                gch = ewp.tile([32, HID], F32, tag="gch")
                nc.scalar.activation(gch[:Bd, :], pg[:Bd, GATES:NTOT], AF.Tanh)
                t1 = ewp.tile([32, HID], F32, tag="t1")
                cx_ap = prev_cx[:Bd, :] if t > 0 else zcx[:Bd, :]
                nc.vector.tensor_tensor(
                    t1[:Bd, :], sifo[:Bd, HID : 2 * HID], cx_ap, OP.mult
                )
                t2 = ewp.tile([32, HID], F32, tag="t2")
                nc.vector.tensor_tensor(
                    t2[:Bd, :], sifo[:Bd, 0:HID], gch[:Bd, :], OP.mult
                )
                cy = cxp.tile([32, HID], F32, tag="cx")
                nc.vector.tensor_tensor(cy[:Bd, :], t1[:Bd, :], t2[:Bd, :], OP.add)
                ry = ewp.tile([32, HID], F32, tag="ry")
                nc.scalar.activation(ry[:Bd, :], cy[:Bd, :], AF.Tanh)
                ssum = ewp.tile([32, HID], F32, tag="ssum")
                nc.vector.tensor_tensor(ssum[:Bd, :], ry[:Bd, :], skip_ap, OP.add)
                hy = hyp.tile([32, HID], F32, tag="hy")
                nc.vector.tensor_tensor(
                    hy[:Bd, :], ssum[:Bd, :], sifo[:Bd, 2 * HID : GATES], OP.mult
                )

                # ---- transpose state back: [Bd,256] -> 2x[128,Bd] each ----
                pt = pt_pool.tile([128, 4, 32], F32, tag="pt")
                for i in range(2):
                    nc.tensor.transpose(
                        pt[:, i, :Bd], hy[:Bd, i * 128 : (i + 1) * 128], ident[:Bd, :Bd]
                    )
                    nc.tensor.transpose(
                        pt[:, 2 + i, :Bd],
                        cy[:Bd, i * 128 : (i + 1) * 128],
                        ident[:Bd, :Bd],
                    )
                nc.vector.tensor_copy(hyT_arr[l][:, :, t, :], pt[:, 0:2, :Bd])
                cT_new = cTp.tile([128, 2, 32], BF16, tag="cT")
                nc.vector.tensor_copy(cT_new[:, :, :Bd], pt[:, 2:4, :Bd])

                # ---- outputs ----
                nc.sync.dma_start(hs_out[l][t * Bd : (t + 1) * Bd, :], hy[:Bd, :])

                prev_cT = cT_new
                prev_cx = cy

    _split_multiwaits(nc)
    return nc


def _prep_inputs(x, params, T):
    """Host-side: build per-core input maps."""
    x = np.asarray(x, dtype=np.float32)
    ps = [{k: np.asarray(v, dtype=np.float32) for k, v in p.items()} for p in params]
    wcs = []
    for l, p in enumerate(ps):
        C = NIN if l == 0 else HID
        K = 2 * HID + C + 1
        wc = np.zeros((K, NTOT), dtype=np.float32)
        wc[0:HID, 0:GATES] = p["W_ih"].T
        wc[0:HID, GATES:NTOT] = p["W_hh"].T
        wc[HID : 2 * HID, 0:GATES] = p["W_ic"].T
        wc[2 * HID : 2 * HID + C, 0:GATES] = p["W_ii"].T
        wc[K - 1, 0:GATES] = p["b_ii"] + p["b_ih"] + p["b_ic"]
        wc[K - 1, GATES:NTOT] = p["b_hh"]
        wcs.append(np.ascontiguousarray(wc.astype(ml_dtypes.bfloat16)))
    wirT = np.ascontiguousarray(ps[0]["W_ir"].T.astype(ml_dtypes.bfloat16))

    in_maps = []
    for c in range(NCORES):
        xs = x[:T, c * BPC : (c + 1) * BPC, :]  # [T, 4, 64]
        xT = np.ascontiguousarray(np.transpose(xs, (2, 0, 1)).astype(ml_dtypes.bfloat16))
        m = {"xT0": xT, "wir": wirT}
        for l in range(4):
            m[f"wc{l}"] = wcs[l]
        in_maps.append(m)
    return in_maps


def _assemble(results, T):
    """Gather per-core [Tl*Bd, 256] outputs into the reference's tuple."""
    dims = _dims(T)
    fulls = []
    for l, (r, Tl, Bd) in enumerate(dims):
        full = np.zeros((T, B if T == T_FULL else NCORES * BPC, HID), np.float32)
        for c, res in enumerate(results):
            h = res[f"hs{l}"].reshape(Tl, r, BPC, HID).reshape(Tl * r, BPC, HID)
            full[:, c * BPC : (c + 1) * BPC, :] = h
        fulls.append(full)
    final = fulls[3]
    outs = tuple(fulls[l][-RATES[l] :] for l in range(4))
    return (final, *outs)


_CACHE = {}
LAST_RESULTS = None


def kernel(x, params):
    global LAST_RESULTS
    T = int(os.environ.get("DRNN_T", str(T_FULL)))
    if T not in _CACHE:
        _CACHE[T] = _build_bass(T)
    nc = _CACHE[T]
    in_maps = _prep_inputs(x, params, T)
    trace = os.environ.get("DRNN_TRACE", "0") == "1"
    res = run_bass_kernel_spmd(
        nc, in_maps, core_ids=list(range(NCORES)), trace=trace
    )
    LAST_RESULTS = res
    return _assemble(res.results, T)
